# revision 6
# baseline (speedup 1.0000x reference)
"""MeshConv (GNN message passing) Bass kernel for 8 trn2 NeuronCores.

Strategy
--------
Shard (batch, edge-half): core c handles batch c//2, edge half c%2.
Host marshals per-batch token-major bf16 tables x[b].T (E, 32) and
per-core index lists. Device gathers 5 neighbor tokens per edge with
SWDGE indirect DMA ([128,1]-offset form: one token per partition per
call), combines them on DVE (sums + abs-diffs of the one-ring), PE
transposes token-major G tiles to feature-major and contracts with the
stacked conv weights (K=128 block: [f0, f1+f3, f2+f4, |f1-f3|] and a
K=32 block: |f2-f4|), accumulating in PSUM.  ACT adds bias and the
result is DMAed out as (64, E_half) f32 per core.
"""

import sys

sys.path.insert(0, "/opt/trn_rl_repo")

import numpy as np
import ml_dtypes

B, C_IN, C_OUT, E, KK = 4, 32, 64, 200000, 5
NCORES = 8
EH = E // 2  # edges per core


def _plan(eh, gpc):
    ngroups = -(-eh // 128)
    nchunk = -(-ngroups // gpc)
    edges_pad = nchunk * gpc * 128
    return nchunk, edges_pad


_PROG_CACHE = {}


def _build(table_rows, nchunk, gpc, reps=1, passthrough=False):
    key = (table_rows, nchunk, gpc, reps, passthrough)
    if key in _PROG_CACHE:
        return _PROG_CACHE[key]
    import concourse.bass as bass
    import concourse.bacc as bacc
    import concourse.tile as tile
    from concourse import mybir
    from concourse.masks import make_identity

    dt = mybir.dt
    nc = bacc.Bacc("TRN2", target_bir_lowering=False, debug=False)
    table = nc.dram_tensor("table", [table_rows, C_IN], dt.bfloat16, kind="ExternalInput")
    idx_d = nc.dram_tensor("idx", [nchunk, 128, gpc * 5], dt.int32, kind="ExternalInput")
    wmain_d = nc.dram_tensor("wmain", [128, C_OUT], dt.bfloat16, kind="ExternalInput")
    wd2_d = nc.dram_tensor("wd2", [32, C_OUT], dt.bfloat16, kind="ExternalInput")
    bias_d = nc.dram_tensor("bias", [C_OUT, 1], dt.float32, kind="ExternalInput")
    out_d = nc.dram_tensor("out", [C_OUT, nchunk * gpc * 128], dt.float32, kind="ExternalOutput")

    AT = mybir.AluOpType
    if passthrough:
        # trivial program with identical external I/O — used by test.py to
        # measure the axon dispatch + transfer floor
        with tile.TileContext(nc) as tc:
            with tc.tile_pool(name="pt", bufs=1) as ptp:
                z = ptp.tile([C_OUT, gpc * 128], dt.float32)
                nc.vector.memset(z[:], 0.0)
                for ch in range(nchunk):
                    nc.sync.dma_start(
                        out_d[:, ch * gpc * 128 : (ch + 1) * gpc * 128], z[:]
                    )
        nc.compile()
        _PROG_CACHE[key] = nc
        return nc
    with tile.TileContext(nc) as tc:
        with (
            tc.tile_pool(name="const", bufs=1) as cp,
            tc.tile_pool(name="idxp", bufs=3) as ip,
            tc.tile_pool(name="tsp", bufs=2) as tsp,
            tc.tile_pool(name="ftp", bufs=2) as ftp,
            tc.tile_pool(name="scp", bufs=2) as scp,
            tc.tile_pool(name="rhsp", bufs=6) as rp,
            tc.tile_pool(name="obp", bufs=2) as obp,
            tc.tile_pool(name="psp", bufs=2, space="PSUM") as pp,
        ):
            wmain = cp.tile([128, C_OUT], dt.bfloat16)
            nc.sync.dma_start(wmain[:], wmain_d[:])
            wd2 = cp.tile([32, C_OUT], dt.bfloat16)
            nc.sync.dma_start(wd2[:], wd2_d[:])
            bias = cp.tile([C_OUT, 1], dt.float32)
            nc.sync.dma_start(bias[:], bias_d[:])
            ident = cp.tile([128, 128], dt.bfloat16)
            make_identity(nc, ident[:])

            for _ in range(reps):
                for ch in range(nchunk):
                    it = ip.tile([128, gpc * 5], dt.int32, tag="it")
                    nc.sync.dma_start(it[:], idx_d[ch])
                    ts = tsp.tile([128, gpc, 160], dt.bfloat16, tag="ts")
                    ft = ftp.tile([128, gpc, 128], dt.bfloat16, tag="ft")
                    for g in range(gpc):
                        nc.gpsimd.indirect_dma_start(
                            out=ts[:, g, 0:32],
                            out_offset=None,
                            in_=table[:],
                            in_offset=bass.IndirectOffsetOnAxis(ap=it[:, g * 5 : g * 5 + 1], axis=0),
                        )
                        for j in range(1, 5):
                            nc.gpsimd.indirect_dma_start(
                                out=ft[:, g, (j - 1) * 32 : j * 32],
                                out_offset=None,
                                in_=table[:],
                                in_offset=bass.IndirectOffsetOnAxis(
                                    ap=it[:, g * 5 + j : g * 5 + j + 1], axis=0
                                ),
                            )
                    f1 = ft[:, :, 0:32]
                    f2 = ft[:, :, 32:64]
                    f3 = ft[:, :, 64:96]
                    f4 = ft[:, :, 96:128]
                    sc1 = scp.tile([128, gpc, 32], dt.bfloat16, tag="sc1")
                    sc2 = scp.tile([128, gpc, 32], dt.bfloat16, tag="sc2")
                    nc.vector.tensor_tensor(out=ts[:, :, 32:64], in0=f1, in1=f3, op=AT.add)
                    nc.vector.tensor_tensor(out=ts[:, :, 64:96], in0=f2, in1=f4, op=AT.add)
                    nc.vector.tensor_tensor(out=sc1[:], in0=f1, in1=f3, op=AT.subtract)
                    nc.scalar.activation(
                        ts[:, :, 96:128], sc1[:], mybir.ActivationFunctionType.Abs
                    )
                    nc.vector.tensor_tensor(out=sc2[:], in0=f2, in1=f4, op=AT.subtract)
                    nc.scalar.activation(
                        ts[:, :, 128:160], sc2[:], mybir.ActivationFunctionType.Abs
                    )
                    ob = obp.tile([C_OUT, gpc * 128], dt.float32, tag="ob")
                    for g in range(gpc):
                        t1 = pp.tile([128, 128], dt.bfloat16, tag="t1")
                        nc.tensor.transpose(t1[:], ts[:, g, 0:128], ident[:])
                        r1 = rp.tile([128, 128], dt.bfloat16, tag="r1")
                        nc.any.tensor_copy(r1[:], t1[:])
                        t2 = pp.tile([32, 128], dt.bfloat16, tag="t2")
                        nc.tensor.transpose(t2[:], ts[:, g, 128:160], ident[:])
                        r2 = rp.tile([32, 128], dt.bfloat16, tag="r2")
                        nc.any.tensor_copy(r2[:], t2[:])
                        o = pp.tile([C_OUT, 128], dt.float32, tag="o")
                        nc.tensor.matmul(o[:], wmain[:], r1[:], start=True, stop=False)
                        nc.tensor.matmul(o[:], wd2[:], r2[:], start=False, stop=True)
                        nc.vector.tensor_scalar(
                            out=ob[:, g * 128 : (g + 1) * 128],
                            in0=o[:],
                            scalar1=bias[:],
                            scalar2=None,
                            op0=AT.add,
                        )
                    nc.sync.dma_start(out_d[:, ch * gpc * 128 : (ch + 1) * gpc * 128], ob[:])
    nc.compile()
    _PROG_CACHE[key] = nc
    return nc


def _marshal_core(x_b, gi_core, eh, nchunk, gpc):
    """Per-core inputs: token-major bf16 table + chunked index tile."""
    edges_pad = nchunk * gpc * 128
    gi_pad = np.zeros((edges_pad, 5), np.int32)
    gi_pad[:eh] = gi_core
    idx = (
        gi_pad.reshape(nchunk, gpc, 128, 5)
        .transpose(0, 2, 1, 3)
        .reshape(nchunk, 128, gpc * 5)
    )
    table = np.ascontiguousarray(x_b.T).astype(ml_dtypes.bfloat16)
    return table, np.ascontiguousarray(idx)


def _marshal_weights(W, b):
    Wk = np.asarray(W)[:, :, 0, :]  # (C_OUT, C_IN, 5)
    wmain = np.zeros((128, C_OUT), np.float32)
    for k in range(4):
        wmain[32 * k : 32 * (k + 1), :] = Wk[:, :, k].T
    wd2 = np.ascontiguousarray(Wk[:, :, 4].T)
    bias = np.asarray(b).reshape(C_OUT, 1).astype(np.float32)
    return (
        wmain.astype(ml_dtypes.bfloat16),
        wd2.astype(ml_dtypes.bfloat16),
        bias,
    )


def _run(x, Gi, W, b, gpc=16, reps=1, passthrough=False):
    from concourse.bass_utils import run_bass_kernel_spmd

    x = np.asarray(x)
    Gi = np.asarray(Gi)
    nchunk, _ = _plan(EH, gpc)
    nc = _build(E, nchunk, gpc, reps, passthrough)
    wmain, wd2, bias = _marshal_weights(W, b)
    tables = {}
    in_maps = []
    for c in range(NCORES):
        bb, h = divmod(c, 2)
        if bb not in tables:
            tables[bb] = np.ascontiguousarray(x[bb].T).astype(ml_dtypes.bfloat16)
        gi_core = Gi[bb, h * EH : (h + 1) * EH]
        edges_pad = nchunk * gpc * 128
        gi_pad = np.zeros((edges_pad, 5), np.int32)
        gi_pad[:EH] = gi_core
        idx = (
            gi_pad.reshape(nchunk, gpc, 128, 5)
            .transpose(0, 2, 1, 3)
            .reshape(nchunk, 128, gpc * 5)
        )
        in_maps.append(
            {
                "table": tables[bb],
                "idx": np.ascontiguousarray(idx),
                "wmain": wmain,
                "wd2": wd2,
                "bias": bias,
            }
        )
    res = run_bass_kernel_spmd(nc, in_maps, core_ids=list(range(NCORES)))
    out = np.empty((B, C_OUT, E, 1), np.float32)
    for c in range(NCORES):
        bb, h = divmod(c, 2)
        out[bb, :, h * EH : (h + 1) * EH, 0] = res.results[c]["out"][:, :EH]
    return out


def kernel(x, Gi, W, b):
    return _run(x, Gi, W, b)
